# revision 1
# baseline (speedup 1.0000x reference)
"""nn_ApplyWeights (segment_reduce bilinear gather) on 8 TRN2 NeuronCores.

out[b, p] = sum_k x[b, pix[k, p]] * weight[k, p]
  x: [8, 3145728] f32, weight/pix: [4, 1038240]

Strategy: shard P_OUT across the 8 cores (129,780 outputs each, padded to
131,072). Host transposes x to xT [N_IN, 8] (replicated to every core) so one
gathered row carries all 8 batch values (32B). Device: per output-tile of 128
points x 4 stencil taps, a vector-indirect DMA gathers 128 xT rows (one per
SBUF partition, offsets [128,1] - the hardware contract); vector engine does
the weighted K-reduction; results DMA back in natural order. Host un-shards.
"""
import os, sys, types

sys.path.insert(0, "/opt/trn_rl_repo")
os.environ.setdefault("MYCRO_LOCAL_CACHE", "1")

import numpy as np

# --- make antenv.axon_hooks importable so trace=True profiling works -------
if "antenv.axon_hooks" not in sys.modules:
    _hook_holder = {"h": None}
    _mod = types.ModuleType("antenv.axon_hooks")
    _mod.set_axon_ntff_profile_hook = lambda h: _hook_holder.__setitem__("h", h)
    _mod.get_axon_ntff_profile_hook = lambda: _hook_holder["h"]
    sys.modules["antenv.axon_hooks"] = _mod
    try:
        import antenv

        antenv.axon_hooks = _mod
        from trn_agent_boot.trn_boot import _ntff_profile_via_ctypes

        _h = _ntff_profile_via_ctypes("/opt/axon/libaxon_pjrt.so")
        if _h is not None:
            _mod.set_axon_ntff_profile_hook(_h)
    except Exception:
        pass

from concourse import bacc, bass, tile, mybir
from concourse import bass_utils

bass_utils.upload_artifacts = lambda d: d  # no S3 in this container

# --- problem constants (hardcoded; kernel.py must be self-contained) -------
B = 8
N_IN = 12 * 512 * 512          # 3,145,728
K = 4
P_OUT = 721 * 1440             # 1,038,240
N_CORES = 8
PL = P_OUT // N_CORES          # 129,780 outputs per core
TILE = 128                     # outputs per gather instruction
TPC = 26                       # tiles per chunk
CHUNK_OUT = TILE * TPC         # 3328 outputs per chunk
NCHUNK = 39                    # chunks per core
PLP = CHUNK_OUT * NCHUNK       # 129,792 padded outputs per core (min pad: 12)
COLS = TPC * K                 # 104 gather columns per chunk -> 4056 gathers

_graph_cache = {}


def _build_graph():
    if "nc" in _graph_cache:
        return _graph_cache["nc"]
    nc = bacc.Bacc("TRN2", target_bir_lowering=False, debug=False)
    xT = nc.dram_tensor("xT", [N_IN, B], mybir.dt.float32, kind="ExternalInput").ap()
    offs = nc.dram_tensor(
        "offs", [NCHUNK, 128, COLS], mybir.dt.int32, kind="ExternalInput"
    ).ap()
    wgt = nc.dram_tensor(
        "wgt", [NCHUNK, 128, COLS], mybir.dt.float32, kind="ExternalInput"
    ).ap()
    out = nc.dram_tensor(
        "out", [NCHUNK, 128, TPC * B], mybir.dt.float32, kind="ExternalOutput"
    ).ap()

    with tile.TileContext(nc) as tc:
        with (
            tc.tile_pool(name="io", bufs=4) as io_pool,
            tc.tile_pool(name="g", bufs=4) as g_pool,
            tc.tile_pool(name="o", bufs=4) as o_pool,
        ):
            for c in range(NCHUNK):
                offs_sb = io_pool.tile([128, COLS], mybir.dt.int32)
                nc.sync.dma_start(out=offs_sb[:], in_=offs[c])
                w_sb = io_pool.tile([128, COLS], mybir.dt.float32)
                nc.sync.dma_start(out=w_sb[:], in_=wgt[c])

                g = g_pool.tile([128, COLS, B], mybir.dt.float32)
                for col in range(COLS):
                    nc.gpsimd.indirect_dma_start(
                        out=g[:, col, :],
                        out_offset=None,
                        in_=xT[:],
                        in_offset=bass.IndirectOffsetOnAxis(
                            ap=offs_sb[:, col : col + 1], axis=0
                        ),
                    )

                # prod[p, col, b] = g[p, col, b] * w[p, col]
                gap = g[:, :, :]
                w2 = w_sb[:]
                w_bcast = bass.AP(
                    tensor=w2.tensor,
                    offset=w2.offset,
                    ap=[list(w2.ap[0]), list(w2.ap[1]), [0, B]],
                )
                prod = g_pool.tile([128, COLS, B], mybir.dt.float32)
                nc.vector.tensor_tensor(
                    out=prod[:, :, :],
                    in0=gap,
                    in1=w_bcast,
                    op=mybir.AluOpType.mult,
                )

                # reduce over k (stride B within each group of K columns)
                pap = prod[:, :, :]
                pview = bass.AP(
                    tensor=pap.tensor,
                    offset=pap.offset,
                    ap=[list(pap.ap[0]), [K * B, TPC], [1, B], [B, K]],
                )
                outt = o_pool.tile([128, TPC, B], mybir.dt.float32)
                nc.vector.tensor_reduce(
                    out=outt[:, :, :],
                    in_=pview,
                    axis=mybir.AxisListType.X,
                    op=mybir.AluOpType.add,
                )
                nc.sync.dma_start(
                    out=out[c].rearrange("p (j b) -> p j b", b=B), in_=outt[:, :, :]
                )
    nc.compile()
    _graph_cache["nc"] = nc
    return nc


def _prep_inputs(x, weight, pix):
    """Host-side shard/layout prep. Returns list of 8 in_maps."""
    x = np.asarray(x)
    weight = np.asarray(weight, dtype=np.float32)
    pix = np.asarray(pix)
    xT = np.ascontiguousarray(x.T.astype(np.float32, copy=False))  # [N_IN, B]

    in_maps = []
    for c in range(N_CORES):
        lo = c * PL
        pg = np.zeros((K, PLP), dtype=np.int32)
        wg = np.zeros((K, PLP), dtype=np.float32)
        pg[:, :PL] = pix[:, lo : lo + PL].astype(np.int32)
        wg[:, :PL] = weight[:, lo : lo + PL]
        # offs[chunk, part, j*K + k] = pix[k, ((chunk*TPC + j)*128 + part)]
        # reshape: [K, NCHUNK, TPC, 128] -> [NCHUNK, 128, TPC, K]
        pg4 = pg.reshape(K, NCHUNK, TPC, 128).transpose(1, 3, 2, 0)
        wg4 = wg.reshape(K, NCHUNK, TPC, 128).transpose(1, 3, 2, 0)
        in_maps.append(
            {
                "xT": xT,
                "offs": np.ascontiguousarray(pg4.reshape(NCHUNK, 128, COLS)),
                "wgt": np.ascontiguousarray(wg4.reshape(NCHUNK, 128, COLS)),
            }
        )
    return in_maps


def _unshard(results):
    """results: list of 8 dicts with 'out' [NCHUNK, 128, TPC*B] -> [B, P_OUT]."""
    out = np.empty((B, P_OUT), dtype=np.float32)
    for c in range(N_CORES):
        oc = results[c]["out"].reshape(NCHUNK, 128, TPC, B)
        # p_local = (chunk*TPC + j)*128 + part -> [chunk, j, part]
        flat = oc.transpose(3, 0, 2, 1).reshape(B, PLP)
        out[:, c * PL : (c + 1) * PL] = flat[:, :PL]
    return out


def _run(x, weight, pix, trace=False):
    nc = _build_graph()
    in_maps = _prep_inputs(x, weight, pix)
    res = bass_utils.run_bass_kernel_spmd(
        nc, in_maps, core_ids=list(range(N_CORES)), trace=trace
    )
    return _unshard(res.results), res


def kernel(x, weight, pix):
    out, _ = _run(x, weight, pix, trace=False)
    return out

